# revision 35
# baseline (speedup 1.0000x reference)
"""Trainium2 Bass kernel: Mistral flash-attention block with mixed-precision KV cache.

Sharding: tensor-parallel over heads across 8 NeuronCores. Core c owns
q-heads 4c..4c+3 and kv-head c. Each head's attention output is AllGathered
separately (4 small collectives that overlap the following heads' compute);
each core computes a 512-wide hidden slice of the output projection and the
host concatenates the slices.

Per-core layout strategy:
  - Scores are computed TRANSPOSED: sT[kv, seq] = K_T(stationary) @ qT(moving),
    so softmax'd weights feed attn@V and o_proj with zero on-chip transposes.
  - exp runs on ACT with a constant -9 bias folded into its affine (cancels in
    the softmax ratio, prevents f16 overflow -> inf*0 = NaN on masked lanes).
  - The softmax denominator is computed by ones-vector matmuls packed 4-at-a-
    time into distinct PE column groups (tile_position), so 4 chunks cost one
    pass; the 4 partial rows are summed by a tiny K=4 matmul.
  - RoPE's half-rotation is a 128x128 +/-1 permutation matmul on the PE (no
    partition-shift DMAs).
  - The int4 quantize-dequantize of the past KV runs on the DVE, overlapped
    with the q/k/v projections; input DMAs split across the SP and ACT HWDGE
    queues so the first projection matmul starts ~2us in.
"""
import os
import numpy as np

N_CORES = 8
QL, HID, NH, NKV, HD, PAST = 512, 4096, 32, 8, 128, 3584
KV = PAST + QL              # 4096
NHC = NH // N_CORES         # 4 q-heads per core
GS = 32
NCH = KV // 128             # 32 kv chunks
NQCH = PAST // 128          # 28 quantized (past) chunks
NPAIR = NCH // 2            # 16 chunk pairs per head
INV_SQRT_HD = float(1.0 / np.sqrt(128.0))
EXP_BIAS = -9.0             # exp(s/sqrt(d) - 9): overflow only at s > 20.1 sigma-wise impossible
MAGIC32 = 12582912.0        # 1.5 * 2**23: adding+subtracting rounds f32 to nearest int

_CACHE = {}


def _build():
    import concourse.tile as tile
    from concourse import bacc, mybir

    f32 = mybir.dt.float32
    f16 = mybir.dt.float16
    AF = mybir.ActivationFunctionType
    AL = mybir.AluOpType

    nc = bacc.Bacc("TRN2", target_bir_lowering=False, debug=False,
                   num_devices=N_CORES)

    HIDT = nc.dram_tensor("hidt", [HID, QL], f16, kind="ExternalInput")
    WQT = nc.dram_tensor("wqt", [HID, NHC * HD], f16, kind="ExternalInput")
    WKT = nc.dram_tensor("wkt", [HID, HD], f16, kind="ExternalInput")
    WVT = nc.dram_tensor("wvt", [HID, HD], f16, kind="ExternalInput")
    WOT = nc.dram_tensor("wot", [NH * HD, QL], f16, kind="ExternalInput")
    PKT = nc.dram_tensor("pkt", [HD, PAST], f32, kind="ExternalInput")
    PV = nc.dram_tensor("pv", [PAST, HD], f32, kind="ExternalInput")
    COST = nc.dram_tensor("cost", [HD, QL], f32, kind="ExternalInput")
    SINT = nc.dram_tensor("sint", [HD, QL], f32, kind="ExternalInput")
    MASKP = nc.dram_tensor("maskp", [128, 4 * QL], f16, kind="ExternalInput")
    ONES = nc.dram_tensor("ones", [HD, HD], f16, kind="ExternalInput")
    ROTM = nc.dram_tensor("rotm", [HD, HD], f16, kind="ExternalInput")
    NGR = PAST // GS
    QMN = {}
    for t in ("k", "v"):
        for w in ("mn", "rs", "sc"):
            QMN[w + t] = nc.dram_tensor(w + t, [128, NGR], f32,
                                        kind="ExternalInput")
    OUT = nc.dram_tensor("out", [QL, QL], f32, kind="ExternalOutput")
    # AllGather split 1/2/1: head 0 alone (absorbs collective warmup early),
    # heads 1+2, then head 3 (small last collective -> short tail)
    AGH = [[0], [1, 2], [3]]
    agin = [nc.dram_tensor(f"agin_{p}", [len(hs) * 128, QL], f16)
            for p, hs in enumerate(AGH)]
    agout = [nc.dram_tensor(f"agout_{p}", [N_CORES * len(hs) * 128, QL], f16,
                            addr_space="Shared") for p, hs in enumerate(AGH)]
    rg = [list(range(N_CORES))]

    with tile.TileContext(nc) as tc:
        pconst_cm = tc.tile_pool(name="pconst", bufs=1)
        pconst = pconst_cm.__enter__()
        kt_all = pconst.tile([128, KV], f16, tag="kt_all")
        v_all = pconst.tile([128, NCH * HD], f16, tag="v_all")
        cosT = pconst.tile([128, QL], f32, tag="cosT")
        sinT = pconst.tile([128, QL], f32, tag="sinT")
        ones = pconst.tile([128, 128], f16, tag="ones")
        rmat = pconst.tile([128, 128], f16, tag="rmat")
        masks = pconst.tile([128, 4 * QL], f16, tag="masks")
        ebias = pconst.tile([128, 1], f32, tag="ebias")
        nc.vector.memset(ebias[:], EXP_BIAS)
        b1536 = pconst.tile([128, 1], f32, tag="b1536")
        nc.vector.memset(b1536[:], 1536.0)

        qt_sb_cm = tc.tile_pool(name="pqt", bufs=1)
        pqt = qt_sb_cm.__enter__()
        qt_sb = [pqt.tile([128, QL], f16, tag=f"qt{h}", name=f"qt_sb{h}")
                 for h in range(NHC)]

        with tc.tile_pool(name="pqdq", bufs=1) as pq, \
             tc.tile_pool(name="pstream", bufs=3) as pstr, \
             tc.tile_pool(name="ptmp", bufs=2) as ptmp, \
             tc.tile_pool(name="ps_qkv", bufs=1, space="PSUM") as ps_qkv, \
             tc.tile_pool(name="ps_rope", bufs=2, space="PSUM") as ps_rope:

            # ---- input DMAs on the ACT queue (SP queue feeds projections) ----
            qscale = {}
            for t in ("k", "v"):
                for w in ("mn", "rs", "sc"):
                    sm = pq.tile([128, NGR], f32, tag=w + t)
                    nc.scalar.dma_start(sm[:], QMN[w + t][:])
                    qscale[w + t] = sm
            pk = pq.tile([128, PAST], f32, tag="pk")
            nc.scalar.dma_start(pk[:], PKT[:])
            pvt = pq.tile([128, PAST], f32, tag="pvt")
            nc.scalar.dma_start(
                pvt[:].rearrange("p (c h) -> p c h", h=HD),
                PV[:].rearrange("(c p) h -> p c h", p=128))
            nc.scalar.dma_start(cosT[:], COST[:])
            nc.scalar.dma_start(sinT[:], SINT[:])
            nc.scalar.dma_start(rmat[:], ROTM[:])
            nc.scalar.dma_start(ones[:], ONES[:])
            nc.scalar.dma_start(masks[:], MASKP[:])

            # ---- qdq of the past KV cache (DVE + ACT round; host scales) ----
            def qdq(src, out_grouped, tg):
                g_in = src.rearrange("p (g i) -> p g i", i=GS)
                mnb = qscale["mn" + tg][:].unsqueeze(2).broadcast_to(
                    (128, NGR, GS))
                rsb = qscale["rs" + tg][:].unsqueeze(2).broadcast_to(
                    (128, NGR, GS))
                scb = qscale["sc" + tg][:].unsqueeze(2).broadcast_to(
                    (128, NGR, GS))
                t32 = pq.tile([128, PAST], f32, tag="t32")
                t32g = t32[:].rearrange("p (g i) -> p g i", i=GS)
                nc.vector.tensor_sub(t32g, g_in, mnb)            # x - mn
                u32 = pq.tile([128, PAST], f32, tag="u32")
                u32g = u32[:].rearrange("p (g i) -> p g i", i=GS)
                nc.vector.tensor_mul(u32g, t32g, rsb)            # u = (x-mn)*rs
                # round(u) on ACT: f16 output of u+1536 snaps to the int grid
                r16 = pq.tile([128, PAST], f16, tag="r16", name=f"r16{tg}")
                nc.scalar.activation(r16[:], u32[:], AF.Identity, bias=b1536[:])
                r16g = r16[:].rearrange("p (g i) -> p g i", i=GS)
                s1 = pq.tile([128, PAST], f32, tag="s1")
                nc.vector.scalar_tensor_tensor(                  # (r'-1536)*scale
                    s1[:].rearrange("p (g i) -> p g i", i=GS),
                    r16g, -1536.0, scb, AL.add, AL.mult)
                nc.vector.tensor_add(out_grouped,                # + mn
                                     s1[:].rearrange("p (g i) -> p g i", i=GS),
                                     mnb)

            qdq(pk[:], kt_all[:, 0:PAST].rearrange("p (g i) -> p g i", i=GS), "k")
            qdq(pvt[:], v_all[:, 0:PAST].rearrange("p (g i) -> p g i", i=GS), "v")

            # ------------- q/k/v projections (PE) -------------
            qt_ps = [ps_qkv.tile([128, QL], f32, tag=f"qps{h}", name=f"qt_ps{h}")
                     for h in range(NHC)]
            kt_ps = ps_qkv.tile([128, QL], f32, tag="kps")
            v_ps = ps_qkv.tile([128, QL], f32, tag="vps")
            NK2 = HID // 256
            wk4 = wv4 = None
            for k2 in range(NK2):
                hid2 = pstr.tile([128, 2, QL], f16, tag="hid")
                nc.sync.dma_start(
                    hid2[:],
                    HIDT[k2 * 256:(k2 + 1) * 256, :].rearrange(
                        "(a p) q -> p a q", p=128))
                wq2 = pstr.tile([128, 2, NHC * HD], f16, tag="wq")
                nc.sync.dma_start(
                    wq2[:],
                    WQT[k2 * 256:(k2 + 1) * 256, :].rearrange(
                        "(a p) q -> p a q", p=128))
                if k2 % 2 == 0:
                    wk4 = pstr.tile([128, 4, HD], f16, tag="wk")
                    nc.sync.dma_start(
                        wk4[:],
                        WKT[k2 * 256:(k2 + 2) * 256, :].rearrange(
                            "(a p) q -> p a q", p=128))
                    wv4 = pstr.tile([128, 4, HD], f16, tag="wv")
                    nc.sync.dma_start(
                        wv4[:],
                        WVT[k2 * 256:(k2 + 2) * 256, :].rearrange(
                            "(a p) q -> p a q", p=128))
                for a in range(2):
                    k = 2 * k2 + a
                    aq = k % 4
                    st, sp = (k == 0), (k == 2 * NK2 - 1)
                    for h in range(NHC):
                        nc.tensor.matmul(qt_ps[h][:],
                                         wq2[:, a, h * 128:(h + 1) * 128],
                                         hid2[:, a, :], start=st, stop=sp)
                    nc.tensor.matmul(kt_ps[:], wk4[:, aq, :], hid2[:, a, :],
                                     start=st, stop=sp)
                    # all four seq-chunk groups share one PSUM bank: only the
                    # first matmul clears it (start=True wipes the WHOLE bank)
                    for s in range(4):
                        mm = nc.tensor.matmul(
                            v_ps[:, s * 128:(s + 1) * 128],
                            hid2[:, a, s * 128:(s + 1) * 128], wv4[:, aq, :],
                            start=(st and s == 0), stop=sp,
                            skip_group_check=True)
                        if st and s == 0:
                            v_mm0 = mm
                        elif st:
                            tile.add_dep_helper(
                                mm.ins, v_mm0.ins, sync=False,
                                reason="bank clear before first writes")

            # new V -> cache chunks 28..31 (one copy, f16 rounding on write)
            nc.vector.tensor_copy(v_all[:, NQCH * HD:NCH * HD], v_ps[:])

            # RoPE: half-rotation via +/-1 permutation matmul on the PE; the
            # PSUM->SBUF staging copy runs on ACT so DVE only does 3 ops/head
            def rope(ps_in, out_ap):
                xsb = ptmp.tile([128, QL], f16, tag="xsb")
                nc.scalar.copy(xsb[:], ps_in)
                rot_ps = ps_rope.tile([128, QL], f32, tag="rot")
                nc.tensor.matmul(rot_ps[:], rmat[:], xsb[:], start=True,
                                 stop=True)
                tcos = ptmp.tile([128, QL], f32, tag="tcos")
                nc.vector.tensor_mul(tcos[:], xsb[:], cosT[:])
                tsin = ptmp.tile([128, QL], f32, tag="tsin")
                nc.vector.tensor_mul(tsin[:], rot_ps[:], sinT[:])
                nc.vector.tensor_add(out_ap, tcos[:], tsin[:])

            for h in range(NHC):
                rope(qt_ps[h][:], qt_sb[h][:])
            rope(kt_ps[:], kt_all[:, PAST:KV])

        # prefetch the whole o_proj weight slice early on the ACT queue (the
        # sync queue is kept free for agin writes / agout reads)
        pwot_cm = tc.tile_pool(name="pwot", bufs=1)
        pwot = pwot_cm.__enter__()
        wot_tiles = []
        for g in range(NH):
            wt = pwot.tile([128, QL], f16, tag=f"wot{g}", name=f"wot{g}")
            nc.scalar.dma_start(wt[:], WOT[g * 128:(g + 1) * 128, :])
            wot_tiles.append(wt)

        # ------------- attention + o_proj, head by head -------------
        with tc.tile_pool(name="pexp", bufs=3) as pexp, \
             tc.tile_pool(name="pmisc", bufs=2) as pmisc, \
             tc.tile_pool(name="pacc", bufs=1) as pacc, \
             tc.tile_pool(name="pag", bufs=2) as pag, \
             tc.tile_pool(name="ps_s", bufs=2, space="PSUM") as ps_s, \
             tc.tile_pool(name="ps_u", bufs=1, space="PSUM") as ps_u, \
             tc.tile_pool(name="ps_d", bufs=1, space="PSUM") as ps_d, \
             tc.tile_pool(name="ps_o", bufs=2, space="PSUM") as ps_o:

            acc = [pacc.tile([128, QL], f32, tag=f"acc{s}", name=f"acc{s}")
                   for s in range(4)]

            # one o_proj part per AG group; nh = heads in the group
            def oproj_part(p, first):
                nh = len(AGH[p])
                ag3d = pag.tile([128, nh * N_CORES, QL], f16,
                                tag=f"ag3d{nh}")
                nc.sync.dma_start(
                    ag3d[:],
                    agout[p][:].rearrange("(b pp) q -> pp b q", pp=128))
                for s in range(4):
                    o_ps = ps_o.tile([128, QL], f32, tag="ops")
                    for b in range(nh * N_CORES):
                        cp, hh = divmod(b, nh)
                        g = cp * NHC + AGH[p][hh]
                        nc.tensor.matmul(o_ps[:],
                                         ag3d[:, b, s * 128:(s + 1) * 128],
                                         wot_tiles[g][:],
                                         start=(b == 0),
                                         stop=(b == nh * N_CORES - 1))
                    if first:
                        nc.vector.tensor_copy(acc[s][:], o_ps[:])
                    else:
                        nc.vector.tensor_add(acc[s][:], acc[s][:], o_ps[:])

            for h in range(NHC):
                outU = ps_u.tile([128, QL], f32, tag="outU", name=f"outU{h}")
                # full ones stationary -> den lands broadcast on all 128
                # partitions, so no cross-partition broadcast is ever needed
                den_ps = ps_d.tile([128, QL], f32, tag="den", name=f"den{h}")
                epairs = [None] * NPAIR
                for j in range(NPAIR + 1):
                    if j < NPAIR:
                        s_ps = ps_s.tile([128, 2 * QL], f32, tag="score",
                                         name=f"s_ps{h}_{j}")
                        for a in range(2):
                            c = 2 * j + a
                            nc.tensor.matmul(
                                s_ps[:, a * QL:(a + 1) * QL],
                                kt_all[:, c * 128:(c + 1) * 128],
                                qt_sb[h][:], start=True, stop=True)
                        e = pexp.tile([128, 2 * QL], f16, tag="e")
                        nc.scalar.activation(e[:], s_ps[:], AF.Exp,
                                             scale=INV_SQRT_HD, bias=ebias[:])
                        if j >= NQCH // 2:
                            off = (j - NQCH // 2) * 2 * QL
                            nc.vector.tensor_mul(
                                e[:], e[:], masks[:, off:off + 2 * QL])
                        epairs[j] = e
                    jj = j - 1
                    if jj >= 0:
                        for a in range(2):
                            c = 2 * jj + a
                            ea = epairs[jj][:, a * QL:(a + 1) * QL]
                            nc.tensor.matmul(outU[:],
                                             v_all[:, c * HD:(c + 1) * HD],
                                             ea, start=(c == 0),
                                             stop=(c == NCH - 1))
                        if jj % 2 == 1:
                            # 4 den matmuls back-to-back: the ones stationary
                            # loads once per group instead of per chunk
                            for c in range(2 * jj - 2, 2 * jj + 2):
                                pj, pa = divmod(c, 2)
                                ea = epairs[pj][:, pa * QL:(pa + 1) * QL]
                                nc.tensor.matmul(den_ps[:], ones[:, :], ea,
                                                 start=(c == 0),
                                                 stop=(c == NCH - 1))

                rden = pmisc.tile([128, QL], f32, tag="rden")
                nc.vector.reciprocal_approx_fast(rden[:], den_ps[:])
                outT = pmisc.tile([128, QL], f16, tag="outT")
                nc.vector.tensor_mul(outT[:], outU[:], rden[:])

                part = 0 if h == 0 else (1 if h <= 2 else 2)
                slot = 0 if h in (0, 1, 3) else 1
                nc.sync.dma_start(
                    agin[part][slot * 128:(slot + 1) * 128, :], outT[:])
                if h != 1:
                    nc.gpsimd.collective_compute(
                        "AllGather", mybir.AluOpType.bypass, replica_groups=rg,
                        ins=[agin[part][:]], outs=[agout[part][:]])

            oproj_part(0, True)
            oproj_part(1, False)
            oproj_part(2, False)

            for s in range(4):
                nc.sync.dma_start(OUT[s * 128:(s + 1) * 128, :], acc[s][:])

        pwot_cm.__exit__(None, None, None)
        qt_sb_cm.__exit__(None, None, None)
        pconst_cm.__exit__(None, None, None)

    nc.compile()
    return nc


def _host_prep(inputs):
    hid = np.asarray(inputs["hidden_states"], dtype=np.float32)[0]   # [512, 4096]
    wq = np.asarray(inputs["wq"], dtype=np.float32)
    wk = np.asarray(inputs["wk"], dtype=np.float32)
    wv = np.asarray(inputs["wv"], dtype=np.float32)
    wo = np.asarray(inputs["wo"], dtype=np.float32)
    pk = np.asarray(inputs["past_key"], dtype=np.float32)[0]         # [8, 3584, 128]
    pv = np.asarray(inputs["past_value"], dtype=np.float32)[0]
    pos = np.asarray(inputs["position_ids"])[0].astype(np.float32)   # [512]

    hidT = np.ascontiguousarray(hid.T)
    inv_freq = np.float32(1.0) / (
        np.float32(10000.0) ** (np.arange(0, HD, 2, dtype=np.float32)
                                / np.float32(HD)))
    freqs = (pos[:, None] * inv_freq[None, :]).astype(np.float32)    # [512, 64]
    emb = np.concatenate([freqs, freqs], axis=-1).astype(np.float64)
    cosT = np.ascontiguousarray(np.cos(emb).astype(np.float32).T)    # [128, 512]
    sinT = np.ascontiguousarray(np.sin(emb).astype(np.float32).T)
    mask = (np.arange(QL)[:, None] <= np.arange(QL)[None, :]).astype(np.float32)
    # device layout: [partition, (mask-chunk, seq)]
    maskp = np.ascontiguousarray(
        mask.reshape(4, 128, QL).transpose(1, 0, 2).reshape(128, 4 * QL)
    ).astype(np.float16)
    ones = np.ones((HD, HD), np.float16)
    # rot(x)[i] = -x[i+64] (i<64), x[i-64] (i>=64) as lhsT[p, i]
    rotm = np.zeros((HD, HD), np.float16)
    rotm[np.arange(64) + 64, np.arange(64)] = np.float16(-1.0)
    rotm[np.arange(64), np.arange(64) + 64] = np.float16(1.0)

    def qstats(xg):
        # xg: [128, 112, 32] f32 groups; reference-f32 min/max/scale semantics
        mn = xg.min(-1)
        mx = xg.max(-1)
        sc = ((mx - mn) / np.float32(15.0)).astype(np.float32)
        rs = (np.float32(1.0) / np.maximum(sc, np.float32(1e-30))).astype(np.float32)
        return (np.ascontiguousarray(mn), np.ascontiguousarray(rs),
                np.ascontiguousarray(sc))

    hidT16 = hidT.astype(np.float16)
    in_maps = []
    for c in range(N_CORES):
        kg = np.ascontiguousarray(pk[c].T).reshape(128, 112, 32)
        mnk, rsk, sck = qstats(kg)
        vg = pv[c].reshape(28, 128, 4, 32).transpose(1, 0, 2, 3).reshape(128, 112, 32)
        mnv, rsv, scv = qstats(vg)
        in_maps.append({
            "mnk": mnk, "rsk": rsk, "sck": sck,
            "mnv": mnv, "rsv": rsv, "scv": scv,
            "hidt": hidT16,
            "wqt": np.ascontiguousarray(wq[c * 512:(c + 1) * 512, :].T).astype(np.float16),
            "wkt": np.ascontiguousarray(wk[c * 128:(c + 1) * 128, :].T).astype(np.float16),
            "wvt": np.ascontiguousarray(wv[c * 128:(c + 1) * 128, :].T).astype(np.float16),
            "wot": np.ascontiguousarray(wo[c * 512:(c + 1) * 512, :].T).astype(np.float16),
            "pkt": np.ascontiguousarray(pk[c].T),
            "pv": np.ascontiguousarray(pv[c]),
            "cost": cosT,
            "sint": sinT,
            "maskp": maskp,
            "ones": ones,
            "rotm": rotm,
        })
    return in_maps


def _run(inputs, trace=False):
    from concourse.bass_utils import run_bass_kernel_spmd
    if "nc" not in _CACHE:
        _CACHE["nc"] = _build()
    nc = _CACHE["nc"]
    in_maps = _host_prep(inputs)
    res = run_bass_kernel_spmd(nc, in_maps, list(range(N_CORES)), trace=trace)
    out = np.concatenate([res.results[c]["out"] for c in range(N_CORES)], axis=1)
    return out.reshape(1, QL, HID).astype(np.float32), res


def kernel(**inputs) -> np.ndarray:
    out, _ = _run(inputs, trace=False)
    return out
